# revision 5
# baseline (speedup 1.0000x reference)
"""Trainium2 Bass kernel for a transformer decoder layer (pre-norm, eval mode).

Computation (per batch row):
    x = x + MHA(LN1(x), LN1(x), LN1(x), mask)      # masked self-attention
    x = x + MHA(LN2(x), enc, enc, None)            # cross-attention
    x = x + W2 @ relu(W1 @ LN3(x) + b1) + b2       # FFN

Shapes: B=4, S=2048, D=512, H=8 heads (dk=64), FF=1024, fp32.

Sharding: 8 cores = (batch b, query-half). Each core computes 1024 query rows
of one batch, with the full 2048-token K/V context. No collectives needed.

Major layout/engine choices (v2):
  - all matmul operands bf16 (scores QK, attn@V, FFN); accumulation fp32 PSUM
  - x1T/x2T/x3T ([dk, token] layouts for the scores lhsT/rhs) are built with
    DMA-engine XBAR transposes (dma_start_transpose), not PE transposes
  - V packed as [v_h | 1] per head: attn@V emits the softmax denominator as
    PSUM row 64; normalization happens AFTER the per-head transpose back to
    token-major, where the denominator is a per-partition column -> one
    [128, H] reciprocal + broadcast-multiply on DVE (no partition broadcast)
  - attention runs query-half-major: all heads for queries 0:512, then
    512:1024. The first half's merge + LN + next-block transpose work hides
    under the second half's matmul stream, and the next attention block's
    first half only depends on the first merge wave.
  - mask is fp8 (exact for 0/1), multiplied into the exp'd scores on
    DVE/GpSimd (alternating per head to balance engines)
  - input DMAs are staged on the Activation engine DGE queue in need order:
    LN consts, x(query half), mask tile 0, x(context half), rest of mask,
    enc K/V, FFN weights. SP queue carries transposes + output stores.
"""

import functools

import numpy as np

B = 4
S = 2048
D = 512
H = 8
DK = 64
DFF = 1024
Q = 1024  # queries per core
P = 128
TS = S // P  # 16 key tiles
TQ = Q // P  # 8 query tiles
EPS = 1e-5
N_CORES = 8
NH = 512  # query-half width (columns per n2 half)


@functools.lru_cache(maxsize=None)
def _build_graph(a1, a2, a3, use_b1, use_b2, repeat=1, no_dma=False):
    """Build the (SPMD, per-core) Bass graph. aN: apply lnN gamma/beta."""
    from contextlib import ExitStack

    import concourse.bacc as bacc
    import concourse.mybir as mybir
    import concourse.tile as tile
    from concourse.masks import make_identity

    f32 = mybir.dt.float32
    bf16 = mybir.dt.bfloat16
    fp8 = mybir.dt.float8e4
    AF = mybir.ActivationFunctionType
    OP = mybir.AluOpType

    nc = bacc.Bacc("TRN2", target_bir_lowering=False, debug=False)

    xq_d = nc.dram_tensor("xq", [P, TQ, D], f32, kind="ExternalInput")
    xk_d = nc.dram_tensor("xk", [P, TQ, D], f32, kind="ExternalInput")
    encT_d = nc.dram_tensor("encT", [P, 4, S], bf16, kind="ExternalInput")
    encv_d = nc.dram_tensor("encv", [P, TS, H, DK + 1], bf16, kind="ExternalInput")
    maskT_d = nc.dram_tensor("maskT", [P, TS, Q], fp8, kind="ExternalInput")
    w1_d = nc.dram_tensor("w1", [P, 4, DFF], bf16, kind="ExternalInput")
    w2_d = nc.dram_tensor("w2", [P, DFF // P, D], bf16, kind="ExternalInput")
    ln_d = {}
    for i, a in ((1, a1), (2, a2), (3, a3)):
        if a:
            ln_d[i] = (
                nc.dram_tensor(f"ln{i}gr", [P, D], f32, kind="ExternalInput"),
                nc.dram_tensor(f"ln{i}br", [P, D], f32, kind="ExternalInput"),
            )
    if use_b1:
        b1t_d = nc.dram_tensor("b1t", [P, DFF // P], f32, kind="ExternalInput")
    if use_b2:
        b2r_d = nc.dram_tensor("b2r", [P, D], f32, kind="ExternalInput")
    out_d = nc.dram_tensor("out", [P, TQ, D], f32, kind="ExternalOutput")

    with tile.TileContext(nc) as tc, ExitStack() as ctx:
        const = ctx.enter_context(tc.tile_pool(name="const", bufs=1))
        big = ctx.enter_context(tc.tile_pool(name="big", bufs=1))
        work = ctx.enter_context(tc.tile_pool(name="work", bufs=4))
        work3 = ctx.enter_context(tc.tile_pool(name="work3", bufs=4))
        work2 = ctx.enter_context(tc.tile_pool(name="work2", bufs=2))
        pssc = ctx.enter_context(tc.tile_pool(name="pssc", bufs=4, space="PSUM"))
        psacc = ctx.enter_context(tc.tile_pool(name="psacc", bufs=4, space="PSUM"))

        identb = const.tile([P, P], bf16)
        make_identity(nc, identb)
        identb65 = const.tile([DK + 1, DK + 1], bf16)
        make_identity(nc, identb65)
        epst = const.tile([P, 1], f32)
        nc.vector.memset(epst, EPS)
        ones = const.tile([P, 1], f32)
        nc.vector.memset(ones, 1.0)
        ln_sb = {}
        for i, (gd, bd) in ln_d.items():
            g = const.tile([P, D], f32, tag=f"ln{i}g")
            b = const.tile([P, D], f32, tag=f"ln{i}b")
            nc.scalar.dma_start(g, gd.ap())
            nc.scalar.dma_start(b, bd.ap())
            ln_sb[i] = (g, b)
        if use_b1:
            b1t = const.tile([P, DFF // P], f32)
            nc.scalar.dma_start(b1t, b1t_d.ap())
        if use_b2:
            b2r = const.tile([P, D], f32)
            nc.scalar.dma_start(b2r, b2r_d.ap())

        def _emit_iteration():
            # persistent tensors
            x1T = big.tile([P, 4, S], bf16, tag="A")            # LN1(x)^T
            v_self = big.tile([P, TS, H, DK + 1], bf16, tag="B")
            maskT = big.tile([P, TS, Q], fp8, tag="C")
            xbuf = big.tile([P, TQ, D], f32, tag="X")           # evolving queries
            xkbuf = big.tile([P, TQ, D], f32, tag="XK")         # context half
            encT = big.tile([P, 4, S], bf16, tag="ENCT")
            encv = big.tile([P, TS, H, DK + 1], bf16, tag="ENCV")
            x2T = big.tile([P, 4, Q], bf16, tag="X2T")
            x3T = big.tile([P, 4, Q], bf16, tag="X3T")
            # per-half attn outputs + denominators, bf16 [65, H, NH]
            aSB = [
                big.tile([DK + 1, H, NH], bf16, tag=f"ASB{n}", name=f"aSB{n}")
                for n in range(2)
            ]

            if no_dma:
                nc.gpsimd.memset(xbuf[:], 0.01)
                nc.gpsimd.memset(xkbuf[:], 0.01)
                nc.gpsimd.memset(maskT[:].bitcast(bf16), 1.0)
                nc.gpsimd.memset(encT[:], 0.01)
                nc.gpsimd.memset(encv[:], 0.01)
            else:
                # Act-queue DMAs in need order
                nc.scalar.dma_start(xbuf[:, 0:4, :], xq_d.ap()[:, 0:4, :])
                nc.scalar.dma_start(xbuf[:, 4:8, :], xq_d.ap()[:, 4:8, :])
                nc.scalar.dma_start(maskT[:, 0:4], maskT_d.ap()[:, 0:4])
                nc.scalar.dma_start(xkbuf[:, 0:4, :], xk_d.ap()[:, 0:4, :])
                nc.scalar.dma_start(xkbuf[:, 4:8, :], xk_d.ap()[:, 4:8, :])
                nc.scalar.dma_start(maskT[:, 4:16], maskT_d.ap()[:, 4:16])
                nc.scalar.dma_start(encT, encT_d.ap())
                nc.scalar.dma_start(encv, encv_d.ap())
            nc.vector.tensor_copy(
                v_self[:, :, :, DK : DK + 1],
                ones[:, None, None, :].to_broadcast([P, TS, H, 1]),
            )

            def layer_norm_tile(x_t, which, use_act=False):
                """LN of a [P, D] tile (tokens on partitions) -> bf16 tile."""
                xn = work3.tile([P, D], bf16, tag="xn")
                if not use_act:
                    stats = work.tile([P, 6], f32, tag="stats")
                    nc.vector.bn_stats(stats, x_t)
                    mv = work.tile([P, 2], f32, tag="mv")
                    nc.vector.bn_aggr(mv, stats)
                    mu = mv[:, 0:1]
                    var = mv[:, 1:2]
                else:
                    tmpf = work3.tile([P, D], f32, tag="tmpf")
                    sums = work.tile([P, 1], f32, tag="sums")
                    nc.scalar.activation(tmpf, x_t, AF.Copy, accum_out=sums)
                    sumsq = work.tile([P, 1], f32, tag="sumsq")
                    nc.scalar.activation(tmpf, x_t, AF.Square, accum_out=sumsq)
                    mu = work.tile([P, 1], f32, tag="mu")
                    nc.vector.tensor_scalar_mul(mu, sums, 1.0 / D)
                    musq = work.tile([P, 1], f32, tag="musq")
                    nc.vector.tensor_mul(musq, mu, mu)
                    var = work.tile([P, 1], f32, tag="var")
                    nc.vector.tensor_scalar(
                        var, sumsq, scalar1=1.0 / D, scalar2=musq,
                        op0=OP.mult, op1=OP.subtract,
                    )
                rstd = work.tile([P, 1], f32, tag="rstd")
                nc.scalar.activation(rstd, var, AF.Sqrt, bias=epst[:])
                nc.vector.reciprocal(rstd, rstd)
                if which in ln_sb:
                    xnf = work3.tile([P, D], f32, tag="xnf")
                    nc.vector.tensor_scalar(
                        xnf, x_t, scalar1=mu, scalar2=rstd,
                        op0=OP.subtract, op1=OP.mult,
                    )
                    g, b = ln_sb[which]
                    nc.vector.tensor_mul(xnf, xnf, g)
                    nc.vector.tensor_add(xn, xnf, b)
                else:
                    nc.vector.tensor_scalar(
                        xn, x_t, scalar1=mu, scalar2=rstd,
                        op0=OP.subtract, op1=OP.mult,
                    )
                return xn

            # ---- phase 1: LN1 over 16 token tiles; build x1T (DMA-T) + V_self
            for t in range(TS):
                x_t = xbuf[:, t, :] if t < 8 else xkbuf[:, t - 8, :]
                x1_t = layer_norm_tile(x_t, 1, use_act=(t % 2 == 1))
                nc.gpsimd.tensor_copy(
                    v_self[:, t, :, 0:DK],
                    x1_t[:].rearrange("p (h d) -> p h d", h=H),
                )
                nc.sync.dma_start_transpose(
                    x1T[:, :, t * P : (t + 1) * P], x1_t[:]
                )

            def attn_block(kT, vv_all, qT, apply_mask, dstT, ln_which):
                """One attention block; adds into xbuf; builds dstT = LN^T.

                Query-half-major: for each n2 half, all 8 heads stream
                scores->exp->mask->attn@V over the 16 key tiles; then the
                half's merge wave (transpose + normalize + residual + LN +
                DMA-transpose into dstT) runs while the other half (or the
                next block) owns the PE.
                """
                for n2 in range(2):
                    q_sl = slice(n2 * NH, (n2 + 1) * NH)
                    for pair in range(H // 2):
                        accs = []
                        for j in range(2):
                            h = pair * 2 + j
                            accs.append(psacc.tile(
                                [DK + 1, NH], mybir.dt.float32, tag="acc",
                                name=f"acc{j}",
                            ))
                        for s in range(TS):
                            for j in range(2):
                                h = pair * 2 + j
                                hp = (h % 2) * DK
                                hf = h // 2
                                sc = pssc.tile(
                                    [P, NH], mybir.dt.float32, tag="sc",
                                    name=f"sc{j}",
                                )
                                nc.tensor.matmul(
                                    sc,
                                    lhsT=kT[hp : hp + DK, hf, s * P : (s + 1) * P],
                                    rhs=qT[hp : hp + DK, hf, q_sl],
                                    start=True, stop=True,
                                )
                                at = work3.tile([P, NH], bf16, tag="attnT",
                                                name=f"at{j}")
                                nc.scalar.activation(at, sc[:], AF.Exp, scale=0.125)
                                if apply_mask:
                                    eng = nc.vector if (h % 2 == 0) else nc.gpsimd
                                    eng.tensor_mul(at, at, maskT[:, s, q_sl])
                                nc.tensor.matmul(
                                    accs[j],
                                    lhsT=vv_all[:, s, h, :],
                                    rhs=at,
                                    start=(s == 0), stop=(s == TS - 1),
                                )
                        for j in range(2):
                            h = pair * 2 + j
                            if j == 0:
                                nc.vector.tensor_copy(aSB[n2][:, h, :], accs[j][:])
                            else:
                                nc.scalar.copy(aSB[n2][:, h, :], accs[j][:])
                    # merge wave for this half: query tiles n2*4 .. n2*4+3
                    for qq in range(4):
                        qt = n2 * 4 + qq
                        pso = pssc.tile([P, H, DK + 2], bf16, tag="sc")
                        for h in range(H):
                            nc.tensor.transpose(
                                pso[:, h, 0 : DK + 1],
                                aSB[n2][:, h, qq * P : (qq + 1) * P],
                                identb65[:],
                            )
                        rcps = work.tile([P, H], mybir.dt.float32, tag="rcps")
                        nc.vector.reciprocal(rcps, pso[:, :, DK])
                        tmp = work2.tile([P, H, DK], bf16, tag="mtmp")
                        nc.vector.tensor_mul(
                            tmp, pso[:, :, 0:DK],
                            rcps[:, :, None].to_broadcast([P, H, DK]),
                        )
                        nc.vector.tensor_add(
                            xbuf[:, qt],
                            xbuf[:, qt],
                            tmp[:].rearrange("p h d -> p (h d)"),
                        )
                        if dstT is not None:
                            xn = layer_norm_tile(
                                xbuf[:, qt], ln_which, use_act=(qq % 2 == 1))
                            nc.sync.dma_start_transpose(
                                dstT[:, :, qt * P : (qt + 1) * P], xn[:]
                            )

            # ---- phase 2: masked self-attention (merge waves build x2T)
            attn_block(x1T, v_self, x1T, apply_mask=True, dstT=x2T, ln_which=2)

            # ---- phase 3: cross-attention (merge waves build x3T)
            attn_block(encT, encv, x2T, apply_mask=False, dstT=x3T, ln_which=3)

            # ---- phase 4: FFN
            w1sb = big.tile([P, 4, DFF], bf16, tag="A")
            w2sb = big.tile([P, DFF // P, D], bf16, tag="W2")
            if no_dma:
                nc.gpsimd.memset(w1sb[:], 0.01)
                nc.gpsimd.memset(w2sb[:], 0.01)
            else:
                nc.scalar.dma_start(w1sb, w1_d.ap())
                nc.scalar.dma_start(w2sb, w2_d.ap())
            hT = big.tile([P, DFF // P, Q], bf16, tag="C")
            for n2 in range(2):
                for f in range(DFF // P):
                    hps = pssc.tile([P, NH], mybir.dt.float32, tag="sc",
                                    name=f"hps{f % 2}")
                    for ft in range(4):
                        nc.tensor.matmul(
                            hps,
                            lhsT=w1sb[:, ft, f * P : (f + 1) * P],
                            rhs=x3T[:, ft, n2 * NH : (n2 + 1) * NH],
                            start=(ft == 0), stop=(ft == 3),
                        )
                    bias = b1t[:, f : f + 1] if use_b1 else 0.0
                    nc.vector.tensor_scalar(
                        hT[:, f, n2 * NH : (n2 + 1) * NH], hps[:],
                        scalar1=bias, scalar2=0.0, op0=OP.add, op1=OP.max,
                    )
            for qt in range(TQ):
                ops = pssc.tile([P, D], mybir.dt.float32, tag="sc")
                for f in range(DFF // P):
                    nc.tensor.matmul(
                        ops,
                        lhsT=hT[:, f, qt * P : (qt + 1) * P],
                        rhs=w2sb[:, f, :],
                        start=(f == 0), stop=(f == DFF // P - 1),
                    )
                nc.vector.tensor_add(xbuf[:, qt], xbuf[:, qt], ops)
                if use_b2:
                    nc.vector.tensor_add(xbuf[:, qt], xbuf[:, qt], b2r)
                nc.sync.dma_start(out_d.ap()[:, qt], xbuf[:, qt])

        if repeat == 1:
            _emit_iteration()
        else:
            with tc.For_i(0, repeat, 1):
                _emit_iteration()

    nc.compile()
    return nc


def _tile_p(a, inner=P):
    """[N*P, ...] -> [P, N, ...] so each SBUF partition's data is contiguous."""
    return np.ascontiguousarray(
        a.reshape(a.shape[0] // inner, inner, *a.shape[1:]).swapaxes(0, 1)
    )


def _prep_core_inputs(x, encoder_output, mask, W1, b1, W2, b2, ln_aff, flags):
    """Build per-core in_maps (host-side sharding + layout prep)."""
    import ml_dtypes

    a1, a2, a3, use_b1, use_b2 = flags
    bf16 = ml_dtypes.bfloat16
    fp8 = ml_dtypes.float8_e4m3
    in_maps = []
    for c in range(N_CORES):
        b, half = c // 2, c % 2
        q0 = half * Q
        perm = np.concatenate(
            [np.arange(q0, q0 + Q), np.arange((1 - half) * Q, (1 - half) * Q + Q)]
        )
        xb = np.ascontiguousarray(x[b][perm]).astype(np.float32)
        enc = encoder_output[b].astype(np.float32)
        encT = np.ascontiguousarray(enc.T).astype(bf16)
        encv = np.empty((S, H, DK + 1), bf16)
        encv[:, :, :DK] = enc.reshape(S, H, DK).astype(bf16)
        encv[:, :, DK] = 1.0
        m = mask[b, 0][q0 : q0 + Q][:, perm]  # [Q, S] in permuted key order
        maskT = np.ascontiguousarray(m.T).astype(fp8)
        im = {
            "xq": _tile_p(xb[0:Q]),
            "xk": _tile_p(xb[Q : 2 * Q]),
            "encT": _tile_p(encT),
            "encv": _tile_p(encv),
            "maskT": _tile_p(maskT),
            "w1": _tile_p(W1.astype(bf16)),
            "w2": _tile_p(W2.astype(bf16)),
        }
        for i, a in ((1, a1), (2, a2), (3, a3)):
            if a:
                g, bta = ln_aff[i]
                im[f"ln{i}gr"] = np.tile(np.asarray(g, np.float32)[None, :], (P, 1))
                im[f"ln{i}br"] = np.tile(np.asarray(bta, np.float32)[None, :], (P, 1))
        if use_b1:
            im["b1t"] = np.ascontiguousarray(
                np.asarray(b1, np.float32).reshape(DFF // P, P).T
            )
        if use_b2:
            im["b2r"] = np.tile(np.asarray(b2, np.float32)[None, :], (P, 1))
        in_maps.append(im)
    return in_maps


def kernel(x, encoder_output, mask, ln1_g, ln1_b, ln2_g, ln2_b, ln3_g, ln3_b,
           W1, b1, W2, b2):
    from concourse import bass_utils

    x = np.asarray(x)
    encoder_output = np.asarray(encoder_output)
    mask = np.asarray(mask)
    ln = {
        1: (np.asarray(ln1_g), np.asarray(ln1_b)),
        2: (np.asarray(ln2_g), np.asarray(ln2_b)),
        3: (np.asarray(ln3_g), np.asarray(ln3_b)),
    }
    flags = (
        *(not (np.all(ln[i][0] == 1.0) and np.all(ln[i][1] == 0.0)) for i in (1, 2, 3)),
        bool(np.any(np.asarray(b1) != 0.0)),
        bool(np.any(np.asarray(b2) != 0.0)),
    )
    nc = _build_graph(*flags)
    in_maps = _prep_core_inputs(
        x, encoder_output, mask, np.asarray(W1), np.asarray(b1), np.asarray(W2),
        np.asarray(b2), ln, flags,
    )
    res = bass_utils.run_bass_kernel_spmd(nc, in_maps, core_ids=list(range(N_CORES)))
    out = np.empty((B, S, D), np.float32)
    for c in range(N_CORES):
        b, half = c // 2, c % 2
        # out dram layout is [P, TQ, D] -> token-major [Q, D]
        o = res.results[c]["out"].swapaxes(0, 1).reshape(Q, D)
        out[b, half * Q : (half + 1) * Q] = o
    return out


# revision 8
# speedup vs baseline: 1.0101x; 1.0101x over previous
"""Trainium2 Bass kernel for a transformer decoder layer (pre-norm, eval mode).

Computation (per batch row):
    x = x + MHA(LN1(x), LN1(x), LN1(x), mask)      # masked self-attention
    x = x + MHA(LN2(x), enc, enc, None)            # cross-attention
    x = x + W2 @ relu(W1 @ LN3(x) + b1) + b2       # FFN

Shapes: B=4, S=2048, D=512, H=8 heads (dk=64), FF=1024, fp32.

Sharding: 8 cores = (batch b, query-half). Each core computes 1024 query rows
of one batch, with the full 2048-token K/V context. No collectives needed.

Layout/engine choices (v3):
  - all matmul operands bf16; fp32 PSUM accumulation
  - x1T built by PE transposes in phase 1 (keeps the PE ticking / HAM warm);
    x2T/x3T built by DMA XBAR transposes on the otherwise idle SP queue,
    overlapped with the next attention half's matmul stream
  - attention chains process KEY-TILE PAIRS: scores for tiles 2s,2s+1 land in
    one 2-bank PSUM tile, one 1024-wide exp + one 1024-wide mask serve both
    -> half the Scalar/DVE instruction count
  - the Scalar engine runs ONLY Exp activations (no table thrashing); LN
    runs entirely on DVE with rstd = (var+eps)^-0.5 via the pow ALU op
  - V packed as [v_h | 1]: attn@V emits the softmax denominator as PSUM row
    64; normalization happens after the per-head transpose to token-major
    ([128, H] reciprocal + broadcast multiply, all DVE)
  - attention is query-half-major (n2 = queries 0:512 then 512:1024); each
    half's merge wave (transpose + normalize + residual + LN + transpose of
    the next block's queries) is emitted one head-pair late so it executes
    under the following half's matmul stream
  - mask fp8 (exact 0/1); self-attn mask multiplies alternate DVE/GpSimd
"""

import functools

import numpy as np

B = 4
S = 2048
D = 512
H = 8
DK = 64
DFF = 1024
Q = 1024  # queries per core
P = 128
TS = S // P  # 16 key tiles
SP2 = TS // 2  # 8 key-tile pairs
TQ = Q // P  # 8 query tiles
EPS = 1e-5
N_CORES = 8
NH = 512  # query-half width


@functools.lru_cache(maxsize=None)
def _build_graph(a1, a2, a3, use_b1, use_b2, repeat=1, no_dma=False):
    """Build the (SPMD, per-core) Bass graph. aN: apply lnN gamma/beta."""
    from contextlib import ExitStack

    import concourse.bacc as bacc
    import concourse.mybir as mybir
    import concourse.tile as tile
    from concourse.masks import make_identity

    f32 = mybir.dt.float32
    bf16 = mybir.dt.bfloat16
    fp8 = mybir.dt.float8e4
    AF = mybir.ActivationFunctionType
    OP = mybir.AluOpType

    nc = bacc.Bacc("TRN2", target_bir_lowering=False, debug=False)

    xq_d = nc.dram_tensor("xq", [P, TQ, D], f32, kind="ExternalInput")
    xk_d = nc.dram_tensor("xk", [P, TQ, D], f32, kind="ExternalInput")
    encT_d = nc.dram_tensor("encT", [P, 4, S], bf16, kind="ExternalInput")
    encv_d = nc.dram_tensor("encv", [P, TS, H, DK + 1], bf16, kind="ExternalInput")
    maskT_d = nc.dram_tensor("maskT", [P, TS, Q], fp8, kind="ExternalInput")
    w1_d = nc.dram_tensor("w1", [P, 4, DFF], bf16, kind="ExternalInput")
    w2_d = nc.dram_tensor("w2", [P, DFF // P, D], bf16, kind="ExternalInput")
    ln_d = {}
    for i, a in ((1, a1), (2, a2), (3, a3)):
        if a:
            ln_d[i] = (
                nc.dram_tensor(f"ln{i}gr", [P, D], f32, kind="ExternalInput"),
                nc.dram_tensor(f"ln{i}br", [P, D], f32, kind="ExternalInput"),
            )
    if use_b1:
        b1t_d = nc.dram_tensor("b1t", [P, DFF // P], f32, kind="ExternalInput")
    if use_b2:
        b2r_d = nc.dram_tensor("b2r", [P, D], f32, kind="ExternalInput")
    out_d = nc.dram_tensor("out", [P, TQ, D], f32, kind="ExternalOutput")

    with tile.TileContext(nc) as tc, ExitStack() as ctx:
        const = ctx.enter_context(tc.tile_pool(name="const", bufs=1))
        big = ctx.enter_context(tc.tile_pool(name="big", bufs=1))
        work = ctx.enter_context(tc.tile_pool(name="work", bufs=4))
        work3 = ctx.enter_context(tc.tile_pool(name="work3", bufs=4))
        work2 = ctx.enter_context(tc.tile_pool(name="work2", bufs=2))
        pssc = ctx.enter_context(tc.tile_pool(name="pssc", bufs=2, space="PSUM"))
        psacc = ctx.enter_context(tc.tile_pool(name="psacc", bufs=4, space="PSUM"))

        identb = const.tile([P, P], bf16)
        make_identity(nc, identb)
        identb65 = const.tile([DK + 1, DK + 1], bf16)
        make_identity(nc, identb65)
        ln_sb = {}
        for i, (gd, bd) in ln_d.items():
            g = const.tile([P, D], f32, tag=f"ln{i}g")
            b = const.tile([P, D], f32, tag=f"ln{i}b")
            nc.scalar.dma_start(g, gd.ap())
            nc.scalar.dma_start(b, bd.ap())
            ln_sb[i] = (g, b)
        if use_b1:
            b1t = const.tile([P, DFF // P], f32)
            nc.scalar.dma_start(b1t, b1t_d.ap())
        if use_b2:
            b2r = const.tile([P, D], f32)
            nc.scalar.dma_start(b2r, b2r_d.ap())
        onesf = const.tile([P, 1], f32)
        nc.vector.memset(onesf, 1.0)

        def _emit_iteration():
            # persistent tensors
            x1T = big.tile([P, 4, S], bf16, tag="A")            # LN1(x)^T
            v_self = big.tile([P, TS, H, DK + 1], bf16, tag="B")
            maskT = big.tile([P, TS, Q], fp8, tag="C")
            xbuf = big.tile([P, TQ, D], f32, tag="X")           # evolving queries
            xkbuf = big.tile([P, TQ, D], f32, tag="XK")         # context half
            encT = big.tile([P, 4, S], bf16, tag="ENCT")
            encv = big.tile([P, TS, H, DK + 1], bf16, tag="ENCV")
            x2T = big.tile([P, 4, Q], bf16, tag="X2T")
            x3T = big.tile([P, 4, Q], bf16, tag="X3T")
            # per-half attn outputs + denominators, bf16 [65, H, NH]
            aSB = [
                big.tile([DK + 1, H, NH], bf16, tag=f"ASB{n}", name=f"aSB{n}")
                for n in range(2)
            ]

            if no_dma:
                nc.gpsimd.memset(xbuf[:], 0.01)
                nc.gpsimd.memset(xkbuf[:], 0.01)
                nc.gpsimd.memset(maskT[:].bitcast(bf16), 1.0)
                nc.gpsimd.memset(encT[:], 0.01)
                nc.gpsimd.memset(encv[:], 0.01)
            else:
                # Act-queue DMAs in need order
                nc.scalar.dma_start(xbuf[:, 0:4, :], xq_d.ap()[:, 0:4, :])
                nc.scalar.dma_start(xbuf[:, 4:8, :], xq_d.ap()[:, 4:8, :])
                nc.scalar.dma_start(maskT[:, 0:4], maskT_d.ap()[:, 0:4])
                nc.scalar.dma_start(xkbuf[:, 0:4, :], xk_d.ap()[:, 0:4, :])
                nc.scalar.dma_start(xkbuf[:, 4:8, :], xk_d.ap()[:, 4:8, :])
                nc.scalar.dma_start(maskT[:, 4:16], maskT_d.ap()[:, 4:16])
                nc.scalar.dma_start(encT, encT_d.ap())
                nc.scalar.dma_start(encv, encv_d.ap())
            nc.gpsimd.tensor_copy(
                v_self[:, :, :, DK : DK + 1],
                onesf[:, None, None, :].to_broadcast([P, TS, H, 1]),
            )

            def layer_norm_tile(x_t, which, xn_eng=None):
                """LN of a [P, D] f32 tile -> bf16 tile. DVE only (no scalar).

                xn_eng: engine for the final normalize pass (default DVE)."""
                xn = work3.tile([P, D], bf16, tag="xn")
                stats = work.tile([P, 6], f32, tag="stats")
                nc.vector.bn_stats(stats, x_t)
                mv = work.tile([P, 2], f32, tag="mv")
                nc.vector.bn_aggr(mv, stats)
                mu = mv[:, 0:1]
                var = mv[:, 1:2]
                # rstd = (var + eps) ** -0.5 on DVE: bit-trick seed + 1 Newton
                # iteration (rel err <= ~0.2%, below the bf16 rounding floor)
                i32 = mybir.dt.int32
                va = work.tile([P, 1], f32, tag="va")
                nc.vector.tensor_scalar_add(va, var, EPS)
                si = work.tile([P, 1], i32, tag="si")
                nc.vector.tensor_scalar(
                    si, va[:].bitcast(i32), scalar1=1, scalar2=None,
                    op0=OP.arith_shift_right,
                )
                # 0x5f3759df - si  ==  (si ^ -1) + 0x5f3759e0
                nc.vector.tensor_scalar(
                    si, si, scalar1=-1, scalar2=None, op0=OP.bitwise_xor,
                )
                nc.vector.tensor_scalar(
                    si, si, scalar1=0x5F3759E0, scalar2=None, op0=OP.add,
                )
                y0 = si[:].bitcast(f32)
                t2 = work.tile([P, 1], f32, tag="t2")
                nc.vector.tensor_mul(t2, y0, y0)
                nc.vector.tensor_mul(t2, t2, va)
                nc.vector.tensor_scalar(
                    t2, t2, scalar1=-0.5, scalar2=1.5, op0=OP.mult, op1=OP.add,
                )
                rstd = work.tile([P, 1], f32, tag="rstd")
                nc.vector.tensor_mul(rstd, y0, t2)
                eng = xn_eng or nc.vector
                if which in ln_sb:
                    xnf = work3.tile([P, D], f32, tag="xnf")
                    eng.tensor_scalar(
                        xnf, x_t, scalar1=mu, scalar2=rstd,
                        op0=OP.subtract, op1=OP.mult,
                    )
                    g, b = ln_sb[which]
                    eng.tensor_mul(xnf, xnf, g)
                    eng.tensor_add(xn, xnf, b)
                else:
                    eng.tensor_scalar(
                        xn, x_t, scalar1=mu, scalar2=rstd,
                        op0=OP.subtract, op1=OP.mult,
                    )
                return xn

            # ---- phase 1: LN1 over 16 token tiles; PE-transpose into x1T
            for t in range(TS):
                x_t = xbuf[:, t, :] if t < 8 else xkbuf[:, t - 8, :]
                x1_t = layer_norm_tile(
                    x_t, 1, xn_eng=(nc.gpsimd if t % 2 else nc.vector))
                nc.gpsimd.tensor_copy(
                    v_self[:, t, :, 0:DK],
                    x1_t[:].rearrange("p (h d) -> p h d", h=H),
                )
                psT = pssc.tile([P, 4, P], bf16, tag="sc")
                for f in range(4):
                    nc.tensor.transpose(
                        psT[:, f, :], x1_t[:, f * P : (f + 1) * P], identb[:]
                    )
                nc.vector.tensor_copy(x1T[:, :, t * P : (t + 1) * P], psT[:])

            def merge_wave(n2, dstT, ln_which):
                """Transpose+normalize+residual+LN for query tiles of half n2."""
                for qq in range(4):
                    qt = n2 * 4 + qq
                    pso = pssc.tile([P, H, DK + 2], bf16, tag="sc")
                    for h in range(H):
                        nc.tensor.transpose(
                            pso[:, h, 0 : DK + 1],
                            aSB[n2][:, h, qq * P : (qq + 1) * P],
                            identb65[:],
                        )
                    rcps = work.tile([P, H], f32, tag="rcps")
                    nc.vector.reciprocal(rcps, pso[:, :, DK])
                    tmp = work2.tile([P, H, DK], bf16, tag="mtmp")
                    nc.vector.tensor_mul(
                        tmp, pso[:, :, 0:DK],
                        rcps[:, :, None].to_broadcast([P, H, DK]),
                    )
                    nc.vector.tensor_add(
                        xbuf[:, qt], xbuf[:, qt],
                        tmp[:].rearrange("p h d -> p (h d)"),
                    )
                    if dstT is not None:
                        xn = layer_norm_tile(xbuf[:, qt], ln_which)
                        nc.sync.dma_start_transpose(
                            dstT[:, :, qt * P : (qt + 1) * P], xn[:]
                        )

            pending = []  # deferred merge wave

            def attn_block(kT, vv_all, qT, apply_mask, dstT, ln_which):
                """One attention block; adds into xbuf; builds dstT = LN^T."""
                for n2 in range(2):
                    q_sl = slice(n2 * NH, (n2 + 1) * NH)
                    for pair in range(H // 2):
                        if pair == 1 and pending:
                            merge_wave(*pending.pop())
                        accs = []
                        for j in range(2):
                            accs.append(psacc.tile(
                                [DK + 1, NH], f32, tag="acc", name=f"acc{j}",
                            ))
                        for sp in range(SP2):
                            for j in range(2):
                                h = pair * 2 + j
                                hp = (h % 2) * DK
                                hf = h // 2
                                sc = pssc.tile([P, 2, NH], f32, tag="sc",
                                               name=f"sc{j}")
                                for i in range(2):
                                    s = sp * 2 + i
                                    nc.tensor.matmul(
                                        sc[:, i, :],
                                        lhsT=kT[hp : hp + DK, hf,
                                                s * P : (s + 1) * P],
                                        rhs=qT[hp : hp + DK, hf, q_sl],
                                        start=True, stop=True,
                                    )
                                at = work3.tile([P, 2, NH], bf16, tag="attnT",
                                                name=f"at{j}")
                                nc.scalar.activation(at, sc[:], AF.Exp,
                                                     scale=0.125)
                                if apply_mask:
                                    eng = nc.vector if j == 0 else nc.gpsimd
                                    eng.tensor_mul(
                                        at, at, maskT[:, sp * 2 : sp * 2 + 2, q_sl]
                                    )
                                for i in range(2):
                                    s = sp * 2 + i
                                    nc.tensor.matmul(
                                        accs[j],
                                        lhsT=vv_all[:, s, h, :],
                                        rhs=at[:, i, :],
                                        start=(sp == 0 and i == 0),
                                        stop=(sp == SP2 - 1 and i == 1),
                                    )
                        for j in range(2):
                            h = pair * 2 + j
                            nc.vector.tensor_copy(aSB[n2][:, h, :], accs[j][:])
                    pending.append((n2, dstT, ln_which))

            # ---- phase 2: masked self-attention (merge waves build x2T)
            attn_block(x1T, v_self, x1T, apply_mask=True, dstT=x2T, ln_which=2)

            # ---- phase 3: cross-attention (merge waves build x3T)
            attn_block(encT, encv, x2T, apply_mask=False, dstT=x3T, ln_which=3)

            # ---- phase 4: FFN
            merge_wave(*pending.pop())  # last cross merge wave
            w1sb = big.tile([P, 4, DFF], bf16, tag="A")
            w2sb = big.tile([P, DFF // P, D], bf16, tag="W2")
            if no_dma:
                nc.gpsimd.memset(w1sb[:], 0.01)
                nc.gpsimd.memset(w2sb[:], 0.01)
            else:
                nc.scalar.dma_start(w1sb, w1_d.ap())
                nc.scalar.dma_start(w2sb, w2_d.ap())
            hT = big.tile([P, DFF // P, Q], bf16, tag="C")
            for n2 in range(2):
                for f in range(DFF // P):
                    hps = pssc.tile([P, NH], f32, tag="sc", name=f"hps{f % 2}")
                    for ft in range(4):
                        nc.tensor.matmul(
                            hps,
                            lhsT=w1sb[:, ft, f * P : (f + 1) * P],
                            rhs=x3T[:, ft, n2 * NH : (n2 + 1) * NH],
                            start=(ft == 0), stop=(ft == 3),
                        )
                    bias = b1t[:, f : f + 1] if use_b1 else 0.0
                    nc.vector.tensor_scalar(
                        hT[:, f, n2 * NH : (n2 + 1) * NH], hps[:],
                        scalar1=bias, scalar2=0.0, op0=OP.add, op1=OP.max,
                    )
            for qt in range(TQ):
                ops = pssc.tile([P, D], f32, tag="sc")
                for f in range(DFF // P):
                    nc.tensor.matmul(
                        ops,
                        lhsT=hT[:, f, qt * P : (qt + 1) * P],
                        rhs=w2sb[:, f, :],
                        start=(f == 0), stop=(f == DFF // P - 1),
                    )
                nc.vector.tensor_add(xbuf[:, qt], xbuf[:, qt], ops)
                if use_b2:
                    nc.vector.tensor_add(xbuf[:, qt], xbuf[:, qt], b2r)
                nc.sync.dma_start(out_d.ap()[:, qt], xbuf[:, qt])

        if repeat == 1:
            _emit_iteration()
        else:
            with tc.For_i(0, repeat, 1):
                _emit_iteration()

    nc.compile()
    return nc


def _tile_p(a, inner=P):
    """[N*P, ...] -> [P, N, ...] so each SBUF partition's data is contiguous."""
    return np.ascontiguousarray(
        a.reshape(a.shape[0] // inner, inner, *a.shape[1:]).swapaxes(0, 1)
    )


def _prep_core_inputs(x, encoder_output, mask, W1, b1, W2, b2, ln_aff, flags):
    """Build per-core in_maps (host-side sharding + layout prep)."""
    import ml_dtypes

    a1, a2, a3, use_b1, use_b2 = flags
    bf16 = ml_dtypes.bfloat16
    fp8 = ml_dtypes.float8_e4m3
    in_maps = []
    for c in range(N_CORES):
        b, half = c // 2, c % 2
        q0 = half * Q
        perm = np.concatenate(
            [np.arange(q0, q0 + Q), np.arange((1 - half) * Q, (1 - half) * Q + Q)]
        )
        xb = np.ascontiguousarray(x[b][perm]).astype(np.float32)
        enc = encoder_output[b].astype(np.float32)
        encT = np.ascontiguousarray(enc.T).astype(bf16)
        encv = np.empty((S, H, DK + 1), bf16)
        encv[:, :, :DK] = enc.reshape(S, H, DK).astype(bf16)
        encv[:, :, DK] = 1.0
        m = mask[b, 0][q0 : q0 + Q][:, perm]  # [Q, S] in permuted key order
        maskT = np.ascontiguousarray(m.T).astype(fp8)
        im = {
            "xq": _tile_p(xb[0:Q]),
            "xk": _tile_p(xb[Q : 2 * Q]),
            "encT": _tile_p(encT),
            "encv": _tile_p(encv),
            "maskT": _tile_p(maskT),
            "w1": _tile_p(W1.astype(bf16)),
            "w2": _tile_p(W2.astype(bf16)),
        }
        for i, a in ((1, a1), (2, a2), (3, a3)):
            if a:
                g, bta = ln_aff[i]
                im[f"ln{i}gr"] = np.tile(np.asarray(g, np.float32)[None, :], (P, 1))
                im[f"ln{i}br"] = np.tile(np.asarray(bta, np.float32)[None, :], (P, 1))
        if use_b1:
            im["b1t"] = np.ascontiguousarray(
                np.asarray(b1, np.float32).reshape(DFF // P, P).T
            )
        if use_b2:
            im["b2r"] = np.tile(np.asarray(b2, np.float32)[None, :], (P, 1))
        in_maps.append(im)
    return in_maps


def kernel(x, encoder_output, mask, ln1_g, ln1_b, ln2_g, ln2_b, ln3_g, ln3_b,
           W1, b1, W2, b2):
    from concourse import bass_utils

    x = np.asarray(x)
    encoder_output = np.asarray(encoder_output)
    mask = np.asarray(mask)
    ln = {
        1: (np.asarray(ln1_g), np.asarray(ln1_b)),
        2: (np.asarray(ln2_g), np.asarray(ln2_b)),
        3: (np.asarray(ln3_g), np.asarray(ln3_b)),
    }
    flags = (
        *(not (np.all(ln[i][0] == 1.0) and np.all(ln[i][1] == 0.0)) for i in (1, 2, 3)),
        bool(np.any(np.asarray(b1) != 0.0)),
        bool(np.any(np.asarray(b2) != 0.0)),
    )
    nc = _build_graph(*flags)
    in_maps = _prep_core_inputs(
        x, encoder_output, mask, np.asarray(W1), np.asarray(b1), np.asarray(W2),
        np.asarray(b2), ln, flags,
    )
    res = bass_utils.run_bass_kernel_spmd(nc, in_maps, core_ids=list(range(N_CORES)))
    out = np.empty((B, S, D), np.float32)
    for c in range(N_CORES):
        b, half = c // 2, c % 2
        # out dram layout is [P, TQ, D] -> token-major [Q, D]
        o = res.results[c]["out"].swapaxes(0, 1).reshape(Q, D)
        out[b, half * Q : (half + 1) * Q] = o
    return out


# revision 18
# speedup vs baseline: 1.0584x; 1.0478x over previous
"""Trainium2 Bass kernel for a transformer decoder layer (pre-norm, eval mode).

Computation (per batch row):
    x = x + MHA(LN1(x), LN1(x), LN1(x), mask)      # masked self-attention
    x = x + MHA(LN2(x), enc, enc, None)            # cross-attention
    x = x + W2 @ relu(W1 @ LN3(x) + b1) + b2       # FFN

Shapes: B=4, S=2048, D=512, H=8 heads (dk=64), FF=1024, fp32.

Sharding: 8 cores = (batch b, query-half). Each core computes 1024 query rows
of one batch, with the full 2048-token K/V context. No collectives needed.

Layout/engine choices (v3):
  - all matmul operands bf16; fp32 PSUM accumulation
  - x1T built by PE transposes in phase 1 (keeps the PE ticking / HAM warm);
    x2T/x3T built by DMA XBAR transposes on the otherwise idle SP queue,
    overlapped with the next attention half's matmul stream
  - attention chains process KEY-TILE PAIRS: scores for tiles 2s,2s+1 land in
    one 2-bank PSUM tile, one 1024-wide exp + one 1024-wide mask serve both
    -> half the Scalar/DVE instruction count
  - the Scalar engine runs ONLY Exp activations (no table thrashing); LN
    runs entirely on DVE with rstd = (var+eps)^-0.5 via the pow ALU op
  - V packed as [v_h | 1]: attn@V emits the softmax denominator as PSUM row
    64; normalization happens after the per-head transpose to token-major
    ([128, H] reciprocal + broadcast multiply, all DVE)
  - attention is query-half-major (n2 = queries 0:512 then 512:1024); each
    half's merge wave (transpose + normalize + residual + LN + transpose of
    the next block's queries) is emitted one head-pair late so it executes
    under the following half's matmul stream
  - mask fp8 (exact 0/1); self-attn mask multiplies alternate DVE/GpSimd
"""

import functools

import numpy as np

B = 4
S = 2048
D = 512
H = 8
DK = 64
DFF = 1024
Q = 1024  # queries per core
P = 128
TS = S // P  # 16 key tiles
SP2 = TS // 2  # 8 key-tile pairs
TQ = Q // P  # 8 query tiles
EPS = 1e-5
N_CORES = 8
NH = 512  # query-half width


@functools.lru_cache(maxsize=None)
def _build_graph(a1, a2, a3, use_b1, use_b2, repeat=1, no_dma=False):
    """Build the (SPMD, per-core) Bass graph. aN: apply lnN gamma/beta."""
    from contextlib import ExitStack

    import concourse.bacc as bacc
    import concourse.mybir as mybir
    import concourse.tile as tile
    from concourse.masks import make_identity

    f32 = mybir.dt.float32
    bf16 = mybir.dt.bfloat16
    fp8 = mybir.dt.float8e4
    AF = mybir.ActivationFunctionType
    OP = mybir.AluOpType

    nc = bacc.Bacc("TRN2", target_bir_lowering=False, debug=False)

    xq_d = nc.dram_tensor("xq", [P, TQ, D], f32, kind="ExternalInput")
    xk_d = nc.dram_tensor("xk", [P, TQ, D], f32, kind="ExternalInput")
    encT_d = nc.dram_tensor("encT", [DK, H, S], bf16, kind="ExternalInput")
    encv_d = nc.dram_tensor("encv", [P, TS, H, DK + 1], bf16, kind="ExternalInput")
    maskT_d = nc.dram_tensor("maskT", [P, TS, Q], bf16, kind="ExternalInput")
    w1_d = nc.dram_tensor("w1", [P, 4, DFF], bf16, kind="ExternalInput")
    w2_d = nc.dram_tensor("w2", [P, DFF // P, D], bf16, kind="ExternalInput")
    ln_d = {}
    for i, a in ((1, a1), (2, a2), (3, a3)):
        if a:
            ln_d[i] = (
                nc.dram_tensor(f"ln{i}gr", [P, D], f32, kind="ExternalInput"),
                nc.dram_tensor(f"ln{i}br", [P, D], f32, kind="ExternalInput"),
            )
    if use_b1:
        b1t_d = nc.dram_tensor("b1t", [P, DFF // P], f32, kind="ExternalInput")
    if use_b2:
        b2r_d = nc.dram_tensor("b2r", [P, D], f32, kind="ExternalInput")
    out_d = nc.dram_tensor("out", [P, TQ, D], f32, kind="ExternalOutput")

    with tile.TileContext(nc) as tc, ExitStack() as ctx:
        const = ctx.enter_context(tc.tile_pool(name="const", bufs=1))
        big = ctx.enter_context(tc.tile_pool(name="big", bufs=1))
        work = ctx.enter_context(tc.tile_pool(name="work", bufs=4))
        work3 = ctx.enter_context(tc.tile_pool(name="work3", bufs=4))
        work2 = ctx.enter_context(tc.tile_pool(name="work2", bufs=1))
        pssc = ctx.enter_context(tc.tile_pool(name="pssc", bufs=3, space="PSUM"))
        psacc = ctx.enter_context(tc.tile_pool(name="psacc", bufs=2, space="PSUM"))

        identb = const.tile([P, P], bf16)
        make_identity(nc, identb)
        identb65 = const.tile([DK + 1, DK + 1], bf16)
        make_identity(nc, identb65)
        ln_sb = {}
        for i, (gd, bd) in ln_d.items():
            g = const.tile([P, D], f32, tag=f"ln{i}g")
            b = const.tile([P, D], f32, tag=f"ln{i}b")
            nc.scalar.dma_start(g, gd.ap())
            nc.scalar.dma_start(b, bd.ap())
            ln_sb[i] = (g, b)
        if use_b1:
            b1t = const.tile([P, DFF // P], f32)
            nc.scalar.dma_start(b1t, b1t_d.ap())
        if use_b2:
            b2r = const.tile([P, D], f32)
            nc.scalar.dma_start(b2r, b2r_d.ap())
        onesf = const.tile([P, 1], f32)
        nc.vector.memset(onesf, 1.0)
        epst = const.tile([P, 1], f32)
        nc.vector.memset(epst, EPS)

        def _emit_iteration():
            # persistent tensors
            x1T = [
                big.tile([DK, H, NH], bf16, tag=f"A{qd}", name=f"x1T{qd}")
                for qd in range(4)
            ]  # LN1(x)^T in 512-token quadrants, [dim, head, token]
            v_self = big.tile([P, TS, H, DK + 1], bf16, tag="B")
            maskT = big.tile([P, TS, Q], bf16, tag="C")
            xbuf = big.tile([P, TQ, D], f32, tag="X")           # evolving queries
            xkbuf = big.tile([P, TQ, D], f32, tag="XK")         # context half
            encT = big.tile([DK, H, S], bf16, tag="ENCT")
            encv = big.tile([P, TS, H, DK + 1], bf16, tag="ENCV")
            x2T = big.tile([DK, H, Q], bf16, tag="X2T")
            x3T = big.tile([P, 4, Q], bf16, tag="XK")
            # per-half attn outputs + denominators, bf16 [65, H, NH]
            aSB = [
                big.tile([DK + 1, H, NH], bf16, tag=f"ASB{n}", name=f"aSB{n}")
                for n in range(2)
            ]

            if no_dma:
                nc.gpsimd.memset(xbuf[:], 0.01)
                nc.gpsimd.memset(xkbuf[:], 0.01)
                nc.gpsimd.memset(maskT[:], 1.0)
                nc.gpsimd.memset(encT[:], 0.01)
                nc.gpsimd.memset(encv[:], 0.01)
            else:
                # Act-queue DMAs in need order
                nc.scalar.dma_start(xbuf[:, 0:4, :], xq_d.ap()[:, 0:4, :])
                nc.scalar.dma_start(xbuf[:, 4:8, :], xq_d.ap()[:, 4:8, :])
                nc.scalar.dma_start(maskT[:, 0:4], maskT_d.ap()[:, 0:4])
                nc.scalar.dma_start(xkbuf[:, 0:4, :], xk_d.ap()[:, 0:4, :])
                nc.scalar.dma_start(xkbuf[:, 4:8, :], xk_d.ap()[:, 4:8, :])
                nc.scalar.dma_start(maskT[:, 4:16], maskT_d.ap()[:, 4:16])
                nc.scalar.dma_start(encT, encT_d.ap())
                nc.scalar.dma_start(encv, encv_d.ap())
            nc.gpsimd.tensor_copy(
                v_self[:, :, :, DK : DK + 1],
                onesf[:, None, None, :].to_broadcast([P, TS, H, 1]),
            )

            def layer_norm_tile(x_t, which, out_ap=None, in_norm=None,
                                phase1=False):
                """LN of a [P, D] f32 tile -> bf16 (tile or provided AP).

                phase1: scalar engine is idle there, so Sqrt + the normalize
                pass run on Scalar (table cost amortized, no Exp conflicts).
                Otherwise everything stays on DVE (Newton rsqrt, no table
                thrash against the attention Exp stream)."""
                stats = work.tile([P, 6], f32, tag="stats")
                nc.vector.bn_stats(stats, x_t)
                mv = work.tile([P, 2], f32, tag="mv")
                nc.vector.bn_aggr(mv, stats)
                mu = mv[:, 0:1]
                var = mv[:, 1:2]
                rstd = work.tile([P, 1], f32, tag="rstd")
                if phase1:
                    nc.scalar.activation(rstd, var, AF.Sqrt, bias=epst[:])
                    nc.vector.reciprocal(rstd, rstd)
                else:
                    # rstd = (var+eps)**-0.5: bit-trick seed + 1 Newton step
                    i32 = mybir.dt.int32
                    va = work.tile([P, 1], f32, tag="va")
                    nc.vector.tensor_scalar_add(va, var, EPS)
                    si = work.tile([P, 1], i32, tag="si")
                    nc.vector.tensor_scalar(
                        si, va[:].bitcast(i32), scalar1=1, scalar2=None,
                        op0=OP.arith_shift_right,
                    )
                    nc.vector.tensor_scalar(
                        si, si, scalar1=-1, scalar2=None, op0=OP.bitwise_xor,
                    )
                    nc.vector.tensor_scalar(
                        si, si, scalar1=0x5F3759E0, scalar2=None, op0=OP.add,
                    )
                    y0 = si[:].bitcast(f32)
                    t2 = work.tile([P, 1], f32, tag="t2")
                    nc.vector.tensor_mul(t2, y0, y0)
                    nc.vector.tensor_mul(t2, t2, va)
                    nc.vector.tensor_scalar(
                        t2, t2, scalar1=-0.5, scalar2=1.5, op0=OP.mult, op1=OP.add,
                    )
                    nc.vector.tensor_mul(rstd, y0, t2)
                if out_ap is None:
                    xn = work3.tile([P, D], bf16, tag="xn")
                    out_ap = xn[:]
                else:
                    xn = None
                affine = which in ln_sb
                tgt = out_ap
                if affine:
                    tmpn = work3.tile([P, D], f32, tag="tmpn")
                    tgt = tmpn[:]
                x_n = x_t if in_norm is None else in_norm
                if phase1:
                    nmr = work.tile([P, 1], f32, tag="nmr")
                    nc.vector.tensor_scalar(
                        nmr, mu, scalar1=rstd, scalar2=-1.0,
                        op0=OP.mult, op1=OP.mult,
                    )
                    nc.scalar.activation(tgt, x_n, AF.Identity,
                                         bias=nmr[:], scale=rstd[:])
                else:
                    nc.vector.tensor_scalar(
                        tgt, x_n, scalar1=mu, scalar2=rstd,
                        op0=OP.subtract, op1=OP.mult,
                    )
                if affine:
                    g, b = ln_sb[which]
                    nc.vector.tensor_mul(tgt, tgt, g)
                    nc.vector.tensor_add(out_ap, tgt, b)
                return xn

            # ---- phase 1: LN1 -> v_self slots; PE-transpose into x1T quads
            for t in range(TS):
                x_t = xbuf[:, t, :] if t < 8 else xkbuf[:, t - 8, :]
                vslot = v_self[:, t, :, 0:DK]  # [P, H, DK] strided view
                layer_norm_tile(
                    x_t, 1, out_ap=vslot,
                    in_norm=x_t.rearrange("p (h d) -> p h d", h=H),
                    phase1=True,
                )
                psT = pssc.tile([DK, H, P], bf16, tag="sc")
                for h in range(H):
                    nc.tensor.transpose(
                        psT[:, h, :],
                        v_self[:, t, h, 0:DK],
                        identb[:],
                    )
                nc.vector.tensor_copy(
                    x1T[t // 4][:, :, (t % 4) * P : (t % 4 + 1) * P], psT[:]
                )

            def merge_wave(n2, dstT, ln_which):
                """Transpose+normalize+residual+LN for query tiles of half n2."""
                for qq in range(4):
                    qt = n2 * 4 + qq
                    pso = pssc.tile([P, H, DK + 2], bf16, tag="sc")
                    for h in range(H):
                        nc.tensor.transpose(
                            pso[:, h, 0 : DK + 1],
                            aSB[n2][:, h, qq * P : (qq + 1) * P],
                            identb65[:],
                        )
                    rcps = work.tile([P, H], f32, tag="rcps")
                    nc.vector.reciprocal(rcps, pso[:, :, DK])
                    tmp = work2.tile([P, H, DK], bf16, tag="mtmp")
                    nc.vector.tensor_mul(
                        tmp, pso[:, :, 0:DK],
                        rcps[:, :, None].to_broadcast([P, H, DK]),
                    )
                    nc.vector.tensor_add(
                        xbuf[:, qt], xbuf[:, qt],
                        tmp[:].rearrange("p h d -> p (h d)"),
                    )
                    if dstT is not None:
                        xn = layer_norm_tile(xbuf[:, qt], ln_which)
                        nc.sync.dma_start_transpose(
                            dstT[:, :, qt * P : (qt + 1) * P], xn[:]
                        )  # out[p, g, tok] = xn[tok, g*Pout + p]

            pending = []  # deferred merge wave

            def attn_block(kT_fn, vv_all, qT_fn, apply_mask, dstT, ln_which):
                """One attention block; adds into xbuf; builds dstT = LN^T."""
                for n2 in range(2):
                    q_sl = slice(n2 * NH, (n2 + 1) * NH)
                    for pair in range(H // 2):
                        if pair == 1 and pending:
                            merge_wave(*pending.pop())
                        accs = []
                        for j in range(2):
                            accs.append(psacc.tile(
                                [DK + 1, NH], f32, tag="acc", name=f"acc{j}",
                            ))
                        for sp in range(SP2):
                            for j in range(2):
                                h = pair * 2 + j
                                sc = pssc.tile([P, 2, NH], f32, tag="sc",
                                               name=f"sc{j}")
                                for i in range(2):
                                    s = sp * 2 + i
                                    nc.tensor.matmul(
                                        sc[:, i, :],
                                        lhsT=kT_fn(h, s),
                                        rhs=qT_fn(h, n2),
                                        start=True, stop=True,
                                    )
                                at = work3.tile([P, 2, NH], bf16, tag="attnT",
                                                name=f"at{j}")
                                nc.scalar.activation(at, sc[:], AF.Exp,
                                                     scale=0.125)
                                if apply_mask:
                                    nc.vector.tensor_mul(
                                        at, at, maskT[:, sp * 2 : sp * 2 + 2, q_sl]
                                    )
                                for i in range(2):
                                    s = sp * 2 + i
                                    nc.tensor.matmul(
                                        accs[j],
                                        lhsT=vv_all[:, s, h, :],
                                        rhs=at[:, i, :],
                                        start=(sp == 0 and i == 0),
                                        stop=(sp == SP2 - 1 and i == 1),
                                    )
                        for j in range(2):
                            h = pair * 2 + j
                            nc.vector.tensor_copy(aSB[n2][:, h, :], accs[j][:])
                    pending.append((n2, dstT, ln_which))

            # ---- phase 2: masked self-attention (merge waves build x2T)
            attn_block(
                lambda h, s: x1T[s // 4][:, h, (s % 4) * P : (s % 4 + 1) * P],
                v_self,
                lambda h, n2: x1T[n2][:, h, :],
                apply_mask=True, dstT=x2T, ln_which=2)

            # ---- phase 3: cross-attention (merge waves build x3T)
            attn_block(
                lambda h, s: encT[:, h, s * P : (s + 1) * P],
                encv,
                lambda h, n2: x2T[:, h, n2 * NH : (n2 + 1) * NH],
                apply_mask=False, dstT=x3T, ln_which=3)

            # ---- phase 4: FFN
            merge_wave(*pending.pop())  # last cross merge wave
            w1sb = big.tile([P, 4, DFF], bf16, tag="A0")
            w2sb = big.tile([P, DFF // P, D], bf16, tag="A1")
            if no_dma:
                nc.gpsimd.memset(w1sb[:], 0.01)
                nc.gpsimd.memset(w2sb[:], 0.01)
            else:
                nc.scalar.dma_start(w1sb, w1_d.ap())
                nc.scalar.dma_start(w2sb, w2_d.ap())
            hT = big.tile([P, DFF // P, Q], bf16, tag="C")
            for n2 in range(2):
                for f in range(DFF // P):
                    hps = pssc.tile([P, NH], f32, tag="sc", name=f"hps{f % 2}")
                    for ft in range(4):
                        nc.tensor.matmul(
                            hps,
                            lhsT=w1sb[:, ft, f * P : (f + 1) * P],
                            rhs=x3T[:, ft, n2 * NH : (n2 + 1) * NH],
                            start=(ft == 0), stop=(ft == 3),
                        )
                    bias = b1t[:, f : f + 1] if use_b1 else 0.0
                    nc.vector.tensor_scalar(
                        hT[:, f, n2 * NH : (n2 + 1) * NH], hps[:],
                        scalar1=bias, scalar2=0.0, op0=OP.add, op1=OP.max,
                    )
            for qt in range(TQ):
                ops = pssc.tile([P, D], f32, tag="sc")
                for f in range(DFF // P):
                    nc.tensor.matmul(
                        ops,
                        lhsT=hT[:, f, qt * P : (qt + 1) * P],
                        rhs=w2sb[:, f, :],
                        start=(f == 0), stop=(f == DFF // P - 1),
                    )
                nc.vector.tensor_add(xbuf[:, qt], xbuf[:, qt], ops)
                if use_b2:
                    nc.vector.tensor_add(xbuf[:, qt], xbuf[:, qt], b2r)
                nc.sync.dma_start(out_d.ap()[:, qt], xbuf[:, qt])

        if repeat == 1:
            _emit_iteration()
        else:
            with tc.For_i(0, repeat, 1):
                _emit_iteration()

    nc.compile()
    return nc


def _tile_p(a, inner=P):
    """[N*P, ...] -> [P, N, ...] so each SBUF partition's data is contiguous."""
    return np.ascontiguousarray(
        a.reshape(a.shape[0] // inner, inner, *a.shape[1:]).swapaxes(0, 1)
    )


def _prep_core_inputs(x, encoder_output, mask, W1, b1, W2, b2, ln_aff, flags):
    """Build per-core in_maps (host-side sharding + layout prep)."""
    import ml_dtypes

    a1, a2, a3, use_b1, use_b2 = flags
    bf16 = ml_dtypes.bfloat16
    fp8 = ml_dtypes.float8_e4m3
    in_maps = []
    for c in range(N_CORES):
        b, half = c // 2, c % 2
        q0 = half * Q
        perm = np.concatenate(
            [np.arange(q0, q0 + Q), np.arange((1 - half) * Q, (1 - half) * Q + Q)]
        )
        xb = np.ascontiguousarray(x[b][perm]).astype(np.float32)
        enc = encoder_output[b].astype(np.float32)
        encT = np.ascontiguousarray(
            enc.T.reshape(H, DK, S).transpose(1, 0, 2)).astype(bf16)
        encv = np.empty((S, H, DK + 1), bf16)
        encv[:, :, :DK] = enc.reshape(S, H, DK).astype(bf16)
        encv[:, :, DK] = 1.0
        m = mask[b, 0][q0 : q0 + Q][:, perm]  # [Q, S] in permuted key order
        maskT = np.ascontiguousarray(m.T).astype(bf16)
        im = {
            "xq": _tile_p(xb[0:Q]),
            "xk": _tile_p(xb[Q : 2 * Q]),
            "encT": encT,
            "encv": _tile_p(encv),
            "maskT": _tile_p(maskT),
            "w1": _tile_p(W1.astype(bf16)),
            "w2": _tile_p(W2.astype(bf16)),
        }
        for i, a in ((1, a1), (2, a2), (3, a3)):
            if a:
                g, bta = ln_aff[i]
                im[f"ln{i}gr"] = np.tile(np.asarray(g, np.float32)[None, :], (P, 1))
                im[f"ln{i}br"] = np.tile(np.asarray(bta, np.float32)[None, :], (P, 1))
        if use_b1:
            im["b1t"] = np.ascontiguousarray(
                np.asarray(b1, np.float32).reshape(DFF // P, P).T
            )
        if use_b2:
            im["b2r"] = np.tile(np.asarray(b2, np.float32)[None, :], (P, 1))
        in_maps.append(im)
    return in_maps


def kernel(x, encoder_output, mask, ln1_g, ln1_b, ln2_g, ln2_b, ln3_g, ln3_b,
           W1, b1, W2, b2):
    from concourse import bass_utils

    x = np.asarray(x)
    encoder_output = np.asarray(encoder_output)
    mask = np.asarray(mask)
    ln = {
        1: (np.asarray(ln1_g), np.asarray(ln1_b)),
        2: (np.asarray(ln2_g), np.asarray(ln2_b)),
        3: (np.asarray(ln3_g), np.asarray(ln3_b)),
    }
    flags = (
        *(not (np.all(ln[i][0] == 1.0) and np.all(ln[i][1] == 0.0)) for i in (1, 2, 3)),
        bool(np.any(np.asarray(b1) != 0.0)),
        bool(np.any(np.asarray(b2) != 0.0)),
    )
    nc = _build_graph(*flags)
    in_maps = _prep_core_inputs(
        x, encoder_output, mask, np.asarray(W1), np.asarray(b1), np.asarray(W2),
        np.asarray(b2), ln, flags,
    )
    res = bass_utils.run_bass_kernel_spmd(nc, in_maps, core_ids=list(range(N_CORES)))
    out = np.empty((B, S, D), np.float32)
    for c in range(N_CORES):
        b, half = c // 2, c % 2
        # out dram layout is [P, TQ, D] -> token-major [Q, D]
        o = res.results[c]["out"].swapaxes(0, 1).reshape(Q, D)
        out[b, half * Q : (half + 1) * Q] = o
    return out
